# revision 31
# baseline (speedup 1.0000x reference)
"""Trainium2 Bass kernel for GQA attention layer (B=1, T=2048, HID=4096,
32 q-heads / 8 kv-heads, head_dim 128, RoPE, causal) sharded over 8 cores.

Sharding: tensor-parallel over heads. Core c owns q-heads 4c..4c+3 and
kv-head c. Attention outputs (transposed, [512 hd, t]) are AllGathered in
four t-chunks (pipelined against later attention compute); each core then
computes a 512-row slice of the output projection over the full 4096 hd
dims, so no AllReduce is needed. Host assembles the 8 output slices.

Matmuls run in bf16 (PE moving operand streams 2B/cycle, so bf16 is 2x
fp32), accumulation in fp32 PSUM; softmax statistics in fp32.

Schedule: QKV projection for chunks 1-3 is emitted output-major (k chain
first, then q0..q3, v): each head's 32-matmul accumulation chain finishes
early and its RoPE runs on DVE underneath the remaining chains, so the PE
never idles at the proj->attention transition (idle >3.4us re-throttles
the PE clock). x tiles for chunk ti+1 are prefetched into a
double-buffered persistent SBUF generation during chunk ti, keeping proj
phases DMA-quiet. The causal mask reduces to one shared 128x128 triangle
applied in-place to the diagonal 128-col window of each diagonal block.
"""

import numpy as np

import concourse.bacc as bacc
import concourse.mybir as mybir
import concourse.tile as tile
from concourse.bass_utils import run_bass_kernel_spmd
from concourse.tile import add_dep_helper

T = 2048
HID = 4096
D = 128
N_HEADS = 32
N_KV = 8
HQ = N_HEADS // N_KV  # q heads per core (=4)
TT = 512  # t tile
NTT = T // TT  # 4
NH = HID // 128  # 32 h-tiles
SCALE = 1.0 / np.sqrt(np.float32(D))
ROPE_BASE = 10000.0
N_CORES = 8

_F32 = mybir.dt.float32
_DT = mybir.dt.bfloat16

_cached = None


def _build():
    nc = bacc.Bacc("TRN2", target_bir_lowering=False, debug=False, num_devices=N_CORES)

    xT = nc.dram_tensor("xT", [HID, T], _DT, kind="ExternalInput").ap()
    wqkvT = nc.dram_tensor(
        "wqkvT", [HID, (HQ + 2) * D], _DT, kind="ExternalInput"
    ).ap()
    woT = nc.dram_tensor("woT", [HID, HQ * D], _DT, kind="ExternalInput").ap()
    cos2 = nc.dram_tensor("cos2", [128, T], _DT, kind="ExternalInput").ap()
    sinS = nc.dram_tensor("sinS", [128, T], _DT, kind="ExternalInput").ap()
    tri_i = nc.dram_tensor("tri_i", [128, 128], _DT, kind="ExternalInput").ap()
    ones_i = nc.dram_tensor("ones_i", [128, 128], _DT, kind="ExternalInput").ap()
    ident_i = nc.dram_tensor("ident_i", [128, 128], _DT, kind="ExternalInput").ap()
    out = nc.dram_tensor("out", [HQ * D, T], _F32, kind="ExternalOutput").ap()

    Exp = mybir.ActivationFunctionType.Exp

    with tile.TileContext(nc) as tc:
        with (
            tc.tile_pool(name="const", bufs=1) as const,
            tc.tile_pool(name="big", bufs=1) as big,
            tc.tile_pool(name="sb", bufs=1) as sb,
            tc.tile_pool(name="ps", bufs=1, space="PSUM") as ps,
            tc.tile_pool(name="dram", bufs=1, space="DRAM") as dram,
        ):
            # ---- constants / persistent weights in SBUF ----
            cos_sb = const.tile([128, T], _DT, name="cos_sb")
            sin_sb = const.tile([128, T], _DT, name="sin_sb")
            tri_sb = const.tile([128, 128], _DT, name="tri_sb")
            ones_sb = const.tile([128, 128], _DT, name="ones_sb")
            ident_sb = const.tile([128, 128], _DT, name="ident_sb")
            wqkv_t = [
                const.tile([128, (HQ + 2) * D], _DT, name=f"wqkv_t{j}")
                for j in range(NH)
            ]
            wo_sb = const.tile([128, NH * HQ * D], _DT, name="wo_sb")

            nc.gpsimd.dma_start(out=ones_sb[:], in_=ones_i[:])
            nc.gpsimd.dma_start(out=ident_sb[:], in_=ident_i[:])
            # cos/sin/tri are not needed until the first rope (~60us in);
            # their DMAs are emitted inside proj0's loop so the first
            # weight/x transfers get the HBM bandwidth at kernel start

            # two persistent generations of x tiles: gen ti%2 serves chunk ti
            xg = [
                [big.tile([128, TT], _DT, name=f"xg{g}_{j}") for j in range(NH)]
                for g in range(2)
            ]
            # per-chunk q rope outputs, double-buffered by chunk parity
            qrot = [
                [big.tile([128, TT], _DT, name=f"qrot{g}_{h}") for h in range(HQ)]
                for g in range(2)
            ]
            krot = big.tile([128, T], _DT, name="krot")
            v_sb = big.tile([128, T], _DT, name="v_sb")  # V[s,d]: block k at cols 128k

            attn_local = [
                dram.tile([HQ * D, TT], _DT, name=f"attn_local{i}") for i in range(NTT)
            ]
            attn_full = [
                dram.tile(
                    [N_CORES * HQ * D, TT],
                    _DT,
                    addr_space="Shared",
                    name=f"attn_full{i}",
                )
                for i in range(NTT)
            ]

            def x_prefetch(ti):
                g = ti % 2
                tsl = slice(TT * ti, TT * (ti + 1))
                for hi in range(NH):
                    nc.sync.dma_start(
                        out=xg[g][hi][:], in_=xT[128 * hi : 128 * (hi + 1), tsl]
                    )

            def rope(src_ps, dst, ti):
                """dst = src*cos + swap_halves(src)*sin  (src in PSUM fp32).

                DVE multiplies read the PSUM accumulator directly; the
                half-swap goes PSUM->SBUF via DMA (DVE lanes cannot cross
                partitions)."""
                tsl = slice(TT * ti, TT * (ti + 1))
                qf = sb.tile([128, TT], _DT, tag="qf", bufs=3)
                nc.vector.tensor_copy(qf[:], src_ps)
                qs = sb.tile([128, TT], _DT, tag="qs", bufs=2)
                nc.sync.dma_start(out=qs[0:64, :], in_=qf[64:128, :])
                nc.sync.dma_start(out=qs[64:128, :], in_=qf[0:64, :])
                t1 = sb.tile([128, TT], _DT, tag="t1", bufs=3)
                nc.vector.tensor_mul(t1[:], qf[:], cos_sb[:, tsl])
                t2 = sb.tile([128, TT], _DT, tag="t2", bufs=3)
                nc.vector.tensor_mul(t2[:], qs[:], sin_sb[:, tsl])
                return nc.vector.tensor_add(dst, t1[:], t2[:])

            def v_transpose(vT_ps, ti):
                # V: [d, s] -> [s, d] blocks via PE transposes
                vT_sb = sb.tile([128, TT], _DT, tag="vTs", bufs=1)
                nc.vector.tensor_copy(vT_sb[:], vT_ps)
                for j in range(TT // 128):
                    vtr = ps.tile([128, 128], _DT, tag="pp", bufs=2)
                    nc.tensor.transpose(
                        vtr[:], vT_sb[:, 128 * j : 128 * (j + 1)], ident_sb[:]
                    )
                    k = (TT // 128) * ti + j
                    nc.vector.tensor_copy(v_sb[:, 128 * k : 128 * (k + 1)], vtr[:])

            def proj0():
                """Chunk 0: hybrid. First 24 h-tiles hi-major (DMA-paced at
                kernel start: 6 matmuls per arriving weight/x pair keep PE
                ahead of the stream), last 8 as per-output chain segments so
                each output finishes staggered and its rope overlaps the
                remaining segments instead of stalling the PE afterwards."""
                ti = 0
                g = 0
                ph1 = 24
                # q accumulators live as half-slices of the two wide pp
                # tiles; k/v use the projection-chain banks
                qp = [
                    ps.tile([128, 2 * TT], _F32, tag="pp", bufs=2, name=f"qp{i}")
                    for i in range(2)
                ]
                q_ps = [qp[h // 2][:, TT * (h % 2) : TT * (h % 2 + 1)] for h in range(HQ)]
                k_ps = ps.tile([128, TT], _F32, tag="pk", name="k_ps0")[:]
                vT_ps = ps.tile([128, TT], _F32, tag="pv", name="vT_ps0")[:]
                wsl_of = {
                    "k": slice(HQ * D, (HQ + 1) * D),
                    "v": slice((HQ + 1) * D, (HQ + 2) * D),
                }
                acc_of = {"k": k_ps, "v": vT_ps}
                for h in range(HQ):
                    wsl_of[h] = slice(128 * h, 128 * (h + 1))
                    acc_of[h] = q_ps[h]
                for hi in range(NH):
                    hsl = slice(128 * hi, 128 * (hi + 1))
                    nc.sync.dma_start(out=wqkv_t[hi][:], in_=wqkvT[hsl, :])
                    nc.sync.dma_start(out=xg[g][hi][:], in_=xT[hsl, 0:TT])
                    if hi == 8:
                        nc.gpsimd.dma_start(out=cos_sb[:], in_=cos2[:])
                        nc.gpsimd.dma_start(out=sin_sb[:], in_=sinS[:])
                        nc.gpsimd.dma_start(out=tri_sb[:], in_=tri_i[:])
                    if hi >= ph1:
                        continue
                    st = hi == 0
                    for which in ["k", "v", 0, 1, 2, 3]:
                        nc.tensor.matmul(
                            acc_of[which],
                            wqkv_t[hi][:, wsl_of[which]],
                            xg[g][hi][:],
                            start=st,
                            stop=False,
                        )
                for which in ["k", 0, "v", 1, 2, 3]:
                    for hi in range(ph1, NH):
                        nc.tensor.matmul(
                            acc_of[which],
                            wqkv_t[hi][:, wsl_of[which]],
                            xg[g][hi][:],
                            start=False,
                            stop=hi == NH - 1,
                        )
                    if which == "k":
                        rope(k_ps, krot[:, 0:TT], ti)
                    elif which == "v":
                        v_transpose(vT_ps, ti)
                    else:
                        rope(q_ps[which], qrot[0][which][:], ti)

            def projN(ti):
                """Chunks 1-3: output-major. One accumulation chain per
                output (k, q0..q3, v); rope for a head is emitted right
                after its chain so DVE overlaps the later chains."""
                g = ti % 2
                # v runs third so its transpose chain drains on DVE under
                # the q1-q3 chains instead of gating the next attention
                chains = [
                    ("k", slice(HQ * D, (HQ + 1) * D)),
                    (0, slice(0, 128)),
                    ("v", slice((HQ + 1) * D, (HQ + 2) * D)),
                    (1, slice(128, 256)),
                    (2, slice(256, 384)),
                    (3, slice(384, 512)),
                ]
                last_rope = None
                for ci, (which, wsl) in enumerate(chains):
                    # chains alternate between the two projection banks;
                    # the bank's previous chain has been roped away by the
                    # time the next-but-one chain starts
                    acc = ps.tile(
                        [128, TT],
                        _F32,
                        tag="pk" if ci % 2 == 0 else "pv",
                        name=f"acc{ti}_{ci}",
                    )[:]
                    for hi in range(NH):
                        nc.tensor.matmul(
                            acc,
                            wqkv_t[hi][:, wsl],
                            xg[g][hi][:],
                            start=hi == 0,
                            stop=hi == NH - 1,
                        )
                    if which == "k":
                        last_rope = rope(acc, krot[:, TT * ti : TT * (ti + 1)], ti)
                    elif which == "v":
                        v_transpose(acc, ti)
                    else:
                        last_rope = rope(acc, qrot[g][which][:], ti)
                return last_rope

            def wo_load():
                for hi in range(NH):
                    nc.sync.dma_start(
                        out=wo_sb[:, 512 * hi : 512 * (hi + 1)],
                        in_=woT[128 * hi : 128 * (hi + 1), :],
                    )

            def attn(ti):
                g = ti % 2
                nblk = (TT // 128) * (ti + 1)
                npair = nblk // 2
                LAP = 2  # score-pair lookahead to cover exp latency
                seqp = [(h, p) for h in range(HQ) for p in range(npair)]
                attn_ps = {}
                psum_t = {}
                probs_t = {}
                state = {"next": 0, "last": None}

                def lo_of(k):
                    diag = k - (TT // 128) * ti
                    return 128 * diag if diag > 0 else 0

                def emit_pair(h, p):
                    # two scoresT blocks into one wide PSUM tile so a single
                    # ACT call exponentiates both (halves the per-block ACT
                    # overhead, keeping the attention phase PE-bound).
                    # Only the 128-col diagonal window needs masking: cols
                    # below it aren't computed, cols above are all allowed.
                    k0, k1 = 2 * p, 2 * p + 1
                    lo0, lo1 = lo_of(k0), lo_of(k1)
                    scp = ps.tile(
                        [128, 2 * TT],
                        _F32,
                        tag="pp",
                        bufs=2,
                        name=f"scp{h}_{p}",
                    )
                    nc.tensor.matmul(
                        scp[:, lo0:TT],
                        krot[:, 128 * k0 : 128 * (k0 + 1)],
                        qrot[g][h][:, lo0:TT],
                        start=True,
                        stop=True,
                    )
                    nc.tensor.matmul(
                        scp[:, TT + lo1 : 2 * TT],
                        krot[:, 128 * k1 : 128 * (k1 + 1)],
                        qrot[g][h][:, lo1:TT],
                        start=True,
                        stop=True,
                    )
                    probs = sb.tile([128, 2 * TT], _DT, tag="probs", bufs=3)
                    nc.scalar.activation(
                        probs[:, lo0 : 2 * TT], scp[:, lo0 : 2 * TT], Exp,
                        scale=SCALE,
                    )
                    for k, off, lo in ((k0, 0, lo0), (k1, TT, lo1)):
                        if k - (TT // 128) * ti >= 0:
                            nc.vector.tensor_mul(
                                probs[:, off + lo : off + lo + 128],
                                probs[:, off + lo : off + lo + 128],
                                tri_sb[:],
                            )
                    # accumulate the softmax denominator's key sums on DVE so
                    # the PE doesn't pay a ones-matmul per block; one
                    # ones-matmul per head reduces psum over partitions
                    if p == 0:
                        psum_t[h] = sb.tile(
                            [128, TT], _DT, tag="psum", bufs=2, name=f"psum{h}"
                        )
                        nc.vector.tensor_copy(psum_t[h][:], probs[:, 0:TT])
                    else:
                        nc.vector.tensor_add(
                            psum_t[h][:, lo0:TT],
                            psum_t[h][:, lo0:TT],
                            probs[:, lo0:TT],
                        )
                    nc.vector.tensor_add(
                        psum_t[h][:, lo1:TT],
                        psum_t[h][:, lo1:TT],
                        probs[:, TT + lo1 : 2 * TT],
                    )
                    probs_t[(h, p)] = probs

                def pump(pi):
                    while state["next"] < len(seqp) and state["next"] <= pi + LAP:
                        emit_pair(*seqp[state["next"]])
                        state["next"] += 1

                pump(-1)
                for pi, (h, p) in enumerate(seqp):
                    if p == 0:
                        attn_ps[h] = ps.tile(
                            [128, TT], _F32, tag="pa", bufs=2, name=f"attn_ps{h}"
                        )
                    probs = probs_t.pop((h, p))
                    for k, off in ((2 * p, 0), (2 * p + 1, TT)):
                        lo = lo_of(k)
                        nc.tensor.matmul(
                            attn_ps[h][:, lo:TT],
                            v_sb[:, 128 * k : 128 * (k + 1)],
                            probs[:, off + lo : off + TT],
                            start=k == 0,
                            stop=k == nblk - 1,
                        )
                    if p != npair - 1:
                        pump(pi)
                        continue
                    # head finished: preview the next head's scores first so
                    # the den matmul's wait on the DVE add chain doesn't
                    # stall the PE FIFO
                    pump(pi)
                    den_ps = ps.tile(
                        [128, TT], _F32, tag="pp", bufs=2, name=f"den_ps{h}"
                    )
                    nc.tensor.matmul(
                        den_ps[:], ones_sb[:], psum_t[h][:], start=True, stop=True
                    )
                    recip = sb.tile([128, TT], _F32, tag="recip", bufs=2)
                    nc.vector.reciprocal_approx_fast(recip[:], den_ps[:])
                    anorm = sb.tile([128, TT], _DT, tag="anorm", bufs=2)
                    state["last"] = nc.vector.tensor_mul(
                        anorm[:], attn_ps[h][:], recip[:]
                    )
                    nc.sync.dma_start(
                        out=attn_local[ti][128 * h : 128 * (h + 1), :],
                        in_=anorm[:],
                    )
                return state["last"]

            def gather(ti, after=None):
                cc = nc.gpsimd.collective_compute(
                    "AllGather",
                    mybir.AluOpType.bypass,
                    replica_groups=[list(range(N_CORES))],
                    ins=[attn_local[ti].opt()],
                    outs=[attn_full[ti].opt()],
                )
                if after is not None:
                    # delay the gather trigger until the next proj's ropes:
                    # peers have then written their attn_local(ti), so the
                    # collective never sits blocked on a skewed peer while
                    # later DMA traffic queues up behind it
                    add_dep_helper(cc.ins, after.ins, sync=True, reason="ag-delay")

            # the last t-chunk is gathered in two half-gathers (heads 0-1,
            # then 2-3) so the final output projection can start earlier
            attn_half = [
                dram.tile(
                    [N_CORES * 2 * D, TT], _DT, addr_space="Shared", name=f"attn_h{i}"
                )
                for i in range(2)
            ]

            def gather_half(ti, half, after=None):
                cc = nc.gpsimd.collective_compute(
                    "AllGather",
                    mybir.AluOpType.bypass,
                    replica_groups=[list(range(N_CORES))],
                    ins=[attn_local[ti][256 * half : 256 * (half + 1), :]],
                    outs=[attn_half[half].opt()],
                )
                if after is not None:
                    add_dep_helper(cc.ins, after.ins, sync=True, reason="agh-delay")

            def outproj(ti):
                op = [
                    ps.tile(
                        [128, 2 * TT], _F32, tag="pp", bufs=2, name=f"op{ti}_{i}"
                    )
                    for i in range(2)
                ]
                o_ps = [op[o // 2][:, TT * (o % 2) : TT * (o % 2 + 1)] for o in range(4)]
                for hd in range(NH):
                    ag = sb.tile([128, TT], _DT, tag="ag", bufs=8)
                    nc.sync.dma_start(
                        out=ag[:], in_=attn_full[ti][128 * hd : 128 * (hd + 1), :]
                    )
                    st, sp = hd == 0, hd == NH - 1
                    for o in range(4):
                        nc.tensor.matmul(
                            o_ps[o],
                            wo_sb[:, 512 * hd + 128 * o : 512 * hd + 128 * (o + 1)],
                            ag[:],
                            start=st,
                            stop=sp,
                        )
                last = None
                for o in range(4):
                    oc = sb.tile([128, TT], _F32, tag="oc", bufs=2)
                    last = nc.vector.tensor_copy(oc[:], o_ps[o])
                    nc.sync.dma_start(
                        out=out[128 * o : 128 * (o + 1), TT * ti : TT * (ti + 1)],
                        in_=oc[:],
                    )
                return last

            def outproj3():
                op = [
                    ps.tile(
                        [128, 2 * TT], _F32, tag="pp", bufs=2, name=f"op3_{i}"
                    )
                    for i in range(2)
                ]
                o_ps = [op[o // 2][:, TT * (o % 2) : TT * (o % 2 + 1)] for o in range(4)]
                first = True
                for half in range(2):
                    for r in range(N_CORES):
                        for hp in range(2):
                            gidx = 4 * r + 2 * half + hp
                            row = 256 * r + 128 * hp
                            ag = sb.tile([128, TT], _DT, tag="ag", bufs=8)
                            nc.sync.dma_start(
                                out=ag[:], in_=attn_half[half][row : row + 128, :]
                            )
                            sp = half == 1 and r == N_CORES - 1 and hp == 1
                            for o in range(4):
                                nc.tensor.matmul(
                                    o_ps[o],
                                    wo_sb[
                                        :,
                                        512 * gidx + 128 * o : 512 * gidx
                                        + 128 * (o + 1),
                                    ],
                                    ag[:],
                                    start=first,
                                    stop=sp,
                                )
                            first = False
                for o in range(4):
                    oc = sb.tile([128, TT], _F32, tag="oc", bufs=2)
                    nc.vector.tensor_copy(oc[:], o_ps[o])
                    nc.sync.dma_start(
                        out=out[128 * o : 128 * (o + 1), 3 * TT : 4 * TT], in_=oc[:]
                    )

            # schedule: x for chunk ti+1 prefetches behind chunk ti's DMA
            # stream; gathers start as soon as a chunk's attention output
            # is staged (proj phases are DMA-quiet so the AllGather HBM
            # traffic no longer fights the x stream)
            # gathers are triggered only after the NEXT attention phase
            # completes locally: by then every peer (within one phase of
            # skew) has produced its contribution, so the collective never
            # sits waiting on a peer while holding shared DMA resources
            proj0()
            x_prefetch(1)
            attn(0)
            projN(1)
            x_prefetch(2)
            wo_load()
            e1 = attn(1)
            gather(0, after=e1)
            projN(2)
            x_prefetch(3)
            e2 = attn(2)
            gather(1, after=e2)
            outproj(0)
            a3 = projN(3)
            gather(2, after=a3)
            e3 = attn(3)
            gather_half(3, 0, after=e3)
            o1 = outproj(1)
            gather_half(3, 1, after=o1)
            outproj(2)
            outproj3()

    nc.compile()
    return nc


def _host_inputs(hidden_states, Wq, Wk, Wv, Wo):
    import ml_dtypes

    bf16 = ml_dtypes.bfloat16
    x = np.asarray(hidden_states, dtype=np.float32).reshape(T, HID)
    xT = np.ascontiguousarray(x.T).astype(bf16)

    pos = np.arange(T, dtype=np.float32)
    inv_freq = ROPE_BASE ** (-np.arange(0, D, 2, dtype=np.float32) / D)  # [64]
    ang = pos[:, None] * inv_freq[None, :]  # [T, 64]
    cosT = np.cos(ang).T.astype(np.float32)  # [64, T]
    sinT = np.sin(ang).T.astype(np.float32)
    cos2 = np.ascontiguousarray(np.concatenate([cosT, cosT], axis=0))
    sinS = np.ascontiguousarray(np.concatenate([-sinT, sinT], axis=0))

    p = np.arange(128)[:, None]
    c = np.arange(128)[None, :]
    tri = (p <= c).astype(np.float32)
    ones = np.ones((128, 128), dtype=bf16)
    ident = np.eye(128, dtype=np.float32).astype(bf16)

    Wq = np.asarray(Wq, dtype=np.float32)
    Wk = np.asarray(Wk, dtype=np.float32)
    Wv = np.asarray(Wv, dtype=np.float32)
    Wo = np.asarray(Wo, dtype=np.float32)

    in_maps = []
    for cix in range(N_CORES):
        qs = slice(HQ * D * cix, HQ * D * (cix + 1))
        ks = slice(D * cix, D * (cix + 1))
        in_maps.append(
            {
                "xT": xT,
                "wqkvT": np.ascontiguousarray(
                    np.concatenate(
                        [Wq[qs, :].T, Wk[ks, :].T, Wv[ks, :].T], axis=1
                    )
                ).astype(bf16),
                "woT": np.ascontiguousarray(Wo[qs, :].T).astype(bf16),
                "cos2": cos2.astype(bf16),
                "sinS": sinS.astype(bf16),
                "tri_i": tri.astype(bf16),
                "ones_i": ones,
                "ident_i": ident,
            }
        )
    return in_maps


def get_program():
    global _cached
    if _cached is None:
        _cached = _build()
    return _cached


def kernel(hidden_states, Wq, Wk, Wv, Wo):
    nc = get_program()
    in_maps = _host_inputs(hidden_states, Wq, Wk, Wv, Wo)
    res = run_bass_kernel_spmd(nc, in_maps, list(range(N_CORES)))
    outT = np.concatenate([res.results[c]["out"] for c in range(N_CORES)], axis=0)
    return np.ascontiguousarray(outT.T).reshape(1, T, HID).astype(np.float32)
